# revision 1
# baseline (speedup 1.0000x reference)
"""Trainium2 Bass kernel for nn_CrossAttention_4037269258775 (RFA cross-attention).

Math (per batch b):
  q   = query @ W_q.T + b_q                  [T, E] -> view [T, H, D]
  wx  = (q / D**0.25) @ rm[h].T              [T, H, P]
  phi = [sin(wx), cos(wx)] * P**-0.5         [T, H, 2P]
  qs  = phi @ s[b,h]; qz = max(phi @ z[b,h], EPS)
  attn = qs / qz                             [T, E]
  out = attn @ W_out.T + b_out               [T, E]

Sharding: batch b -> core b (B == n_cores == 8). No collectives.

Device dataflow is transposed (feature-major, t on the free dim):
  host precombines M[hp, e] = sum_d rm[h,p,d]/D**0.25 * W_q[h*64+d, e] (fp64)
  wx.T = M @ query_b.T  via error-compensated fp32r (TF32) 3-term split:
         Mr@Xr + Mr@Xe + Me@Xr   (each term 1 cyc/row vs 4 for fp32)
  range-reduce wx on DVE (add_range_wrap x2, +1 more for the cos +pi/2 shift),
  Sin on ACT -> per-head phi tiles [2P=128, Tc]
  fused qs+qz fp32 matmul per head (s_aug has z as column 64, P**-0.5 folded)
  1/max(qz,eps) on DVE; broadcast across 64 partitions via ones[1,64] fp32r
  matmul; attn = qs * recip_bcast on DVE -> fp32r; out-proj fp32r matmul.
"""
import numpy as np
from contextlib import ExitStack

import concourse.bass as bass
import concourse.tile as tile
import concourse.mybir as mybir
from concourse import bacc
from concourse.bass_utils import run_bass_kernel_spmd

dt = mybir.dt

T, B, E = 2048, 8, 1024
H, D, P = 16, 64, 64
EPS = 1e-8
NCORES = 8
TC = 256                      # t-chunk size
NCH = T // TC                 # 8 chunks
NE = E // 128                 # 8 e-tiles (also hp-tiles, e'-tiles, k-tiles)
PI = float(np.pi)
TWO_PI = float(2 * np.pi)
HALF_PI = float(np.pi / 2)

_CACHE = {}


def tf32_round(x):
    u = np.ascontiguousarray(x, np.float32).view(np.uint32)
    r = (u + 0xFFF + ((u >> 13) & 1)) & np.uint32(0xFFFFE000)
    return r.view(np.float32)


def build_kernel(reps=1):
    nc = bacc.Bacc(None, target_bir_lowering=False)

    xtr_d = nc.dram_tensor("xtr", [E, T], dt.float32r, kind="ExternalInput")
    xte_d = nc.dram_tensor("xte", [E, T], dt.float32r, kind="ExternalInput")
    mtr_d = nc.dram_tensor("mtr", [E, E], dt.float32r, kind="ExternalInput")
    mte_d = nc.dram_tensor("mte", [E, E], dt.float32r, kind="ExternalInput")
    wot_d = nc.dram_tensor("wot", [E, E], dt.float32r, kind="ExternalInput")
    saug_d = nc.dram_tensor("saug", [2 * P, H * (D + 1)], dt.float32, kind="ExternalInput")
    # pair-broadcast selectors, one row, free-dim-sliceable: cols 0:128 =
    # [1]*64+[0]*64 (head half 0), cols 128:256 = [0]*64+[1]*64 (half 1)
    ones_d = nc.dram_tensor("ones", [1, 256], dt.float32r, kind="ExternalInput")
    out_d = nc.dram_tensor("out", [E, T], dt.float32, kind="ExternalOutput")

    with tile.TileContext(nc) as tc, ExitStack() as ctx:
        consts = ctx.enter_context(tc.tile_pool(name="consts", bufs=1))
        xtp = ctx.enter_context(tc.tile_pool(name="xtp", bufs=2))
        wrp = ctx.enter_context(tc.tile_pool(name="wrp", bufs=2))
        phip = ctx.enter_context(tc.tile_pool(name="phip", bufs=2))
        rcp = ctx.enter_context(tc.tile_pool(name="rcp", bufs=2))
        attnp = ctx.enter_context(tc.tile_pool(name="attnp", bufs=1))
        outp = ctx.enter_context(tc.tile_pool(name="outp", bufs=2))
        ps_wx = ctx.enter_context(tc.tile_pool(name="ps_wx", bufs=2, space="PSUM"))
        ps_qs = ctx.enter_context(tc.tile_pool(name="ps_qs", bufs=1, space="PSUM"))
        ps_bc = ctx.enter_context(tc.tile_pool(name="ps_bc", bufs=2, space="PSUM"))
        ps_m2 = ctx.enter_context(tc.tile_pool(name="ps_m2", bufs=2, space="PSUM"))

        # ---- constant loads ----
        mtr_t = [consts.tile([128, E], dt.float32r, tag=f"mtr{g}", name=f"mtr{g}") for g in range(NE)]
        mte_t = [consts.tile([128, E], dt.float32r, tag=f"mte{g}", name=f"mte{g}") for g in range(NE)]
        wot_t = [consts.tile([128, E], dt.float32r, tag=f"wot{g}", name=f"wot{g}") for g in range(NE)]
        for g in range(NE):
            nc.sync.dma_start(mtr_t[g][:], mtr_d[128 * g : 128 * (g + 1), :])
            nc.sync.dma_start(mte_t[g][:], mte_d[128 * g : 128 * (g + 1), :])
            nc.sync.dma_start(wot_t[g][:], wot_d[128 * g : 128 * (g + 1), :])
        saug_t = consts.tile([2 * P, H * (D + 1)], dt.float32, tag="saug", name="saug")
        nc.sync.dma_start(saug_t[:], saug_d[:])
        ones_t = consts.tile([1, 256], dt.float32r, tag="ones", name="ones")
        nc.sync.dma_start(ones_t[:], ones_d[:])

        for kk in range(NCH * reps):
            k = kk % NCH
            # ---- streamed X chunk loads (double-buffered per e-tile tag) ----
            xtr_t, xte_t = [], []
            for g in range(NE):
                tr = xtp.tile([128, TC], dt.float32r, tag=f"xtr{g}", name=f"xtr{g}_{k}")
                nc.sync.dma_start(
                    tr[:], xtr_d[128 * g : 128 * (g + 1), TC * k : TC * (k + 1)]
                )
                xtr_t.append(tr)
                te = xtp.tile([128, TC], dt.float32r, tag=f"xte{g}", name=f"xte{g}_{k}")
                nc.sync.dma_start(
                    te[:], xte_d[128 * g : 128 * (g + 1), TC * k : TC * (k + 1)]
                )
                xte_t.append(te)

            attn_t = []
            for i in range(NE):  # hp-tile i: heads 2i (parts 0:64), 2i+1 (64:128)
                # ---- wx = M @ X^T via 3-term fp32r split ----
                wx_ps = ps_wx.tile([128, TC], dt.float32, tag="wx", name=f"wx_{k}_{i}")
                groups = [(mtr_t, xtr_t), (mtr_t, xte_t), (mte_t, xtr_t)]
                n_mm = len(groups) * NE
                mi = 0
                for mg, xg in groups:
                    for g in range(NE):
                        nc.tensor.matmul(
                            wx_ps[:],
                            lhsT=mg[g][:, 128 * i : 128 * (i + 1)],
                            rhs=xg[g][:],
                            start=(mi == 0),
                            stop=(mi == n_mm - 1),
                        )
                        mi += 1
                # ---- range reduction into [-pi, pi] ----
                wr_a = wrp.tile([128, TC], dt.float32, tag="wr_a", name=f"wra_{k}_{i}")
                nc.vector.add_range_wrap(wr_a[:], wx_ps[:], 0.0, PI, TWO_PI)
                wr_s = wrp.tile([128, TC], dt.float32, tag="wr_s", name=f"wrs_{k}_{i}")
                nc.vector.add_range_wrap(wr_s[:], wr_a[:], 0.0, PI, TWO_PI)
                # cos input: one more wrap with +pi/2 shift
                wr_c = wrp.tile([128, TC], dt.float32, tag="wr_c", name=f"wrc_{k}_{i}")
                nc.vector.add_range_wrap(wr_c[:], wr_s[:], HALF_PI, PI, TWO_PI)

                ph = []
                for half in range(2):
                    phi_t = phip.tile(
                        [128, TC], dt.float32, tag=f"phi{half}", name=f"phi_{k}_{i}_{half}"
                    )
                    sl = slice(64 * half, 64 * (half + 1))
                    nc.scalar.activation(
                        phi_t[0:64, :], wr_s[sl, :], mybir.ActivationFunctionType.Sin
                    )
                    nc.scalar.activation(
                        phi_t[64:128, :], wr_c[sl, :], mybir.ActivationFunctionType.Sin
                    )
                    ph.append(phi_t)

                attn_i = attnp.tile(
                    [128, TC], dt.float32r, tag=f"attn{i}", name=f"attn_{k}_{i}"
                )
                qs_pair = []
                rcr = [
                    rcp.tile([1, TC], dt.float32r, tag="rcr0", name=f"rcr0_{k}_{i}"),
                    rcp.tile([1, TC], dt.float32r, tag="rcr1", name=f"rcr1_{k}_{i}"),
                ]
                for half in range(2):
                    h = 2 * i + half
                    # ---- fused qs+qz fp32 matmul: s_aug [128, 65] ----
                    qs_ps = ps_qs.tile(
                        [65, TC], dt.float32, tag=f"qs{half}", name=f"qs_{k}_{h}"
                    )
                    nc.tensor.matmul(
                        qs_ps[:],
                        lhsT=saug_t[:, (D + 1) * h : (D + 1) * (h + 1)],
                        rhs=ph[half][:],
                        start=True,
                        stop=True,
                    )
                    qs_pair.append(qs_ps)
                    # ---- recip of clamped qz (row `half` of the pair tile) ----
                    qz_c = rcp.tile([1, TC], dt.float32, tag="qz_c", name=f"qzc_{k}_{h}", bufs=1)
                    nc.vector.tensor_scalar_max(qz_c[:], qs_ps[64:65, :], EPS)
                    rc32 = rcp.tile([1, TC], dt.float32, tag="rc32", name=f"rc32_{k}_{h}", bufs=1)
                    nc.vector.reciprocal(rc32[:], qz_c[:])
                    nc.vector.tensor_copy(rcr[half][:], rc32[:])
                # ---- broadcast both recips across partitions: two accumulating
                # selector matmuls into one bank ----
                bc_ps = ps_bc.tile([128, TC], dt.float32, tag="bc", name=f"bc_{k}_{i}")
                nc.tensor.matmul(
                    bc_ps[:], lhsT=ones_t[:, 0:128], rhs=rcr[0][:], start=True, stop=False
                )
                nc.tensor.matmul(
                    bc_ps[:], lhsT=ones_t[:, 128:256], rhs=rcr[1][:], start=False, stop=True
                )
                # DVE tensor_tensor allows only one PSUM input: stage bc
                bc_sb = rcp.tile([128, TC], dt.float32, tag="bc_sb", name=f"bcs_{k}_{i}")
                nc.vector.tensor_copy(bc_sb[:], bc_ps[:])
                # ---- attn = qs * recip -> fp32r SBUF ----
                for half in range(2):
                    nc.vector.tensor_mul(
                        attn_i[64 * half : 64 * (half + 1), :],
                        qs_pair[half][0:64, :],
                        bc_sb[64 * half : 64 * (half + 1), :],
                    )
                attn_t.append(attn_i)

            # ---- out projection: fp32r ----
            for j in range(NE):
                m2_ps = ps_m2.tile([128, TC], dt.float32, tag="m2", name=f"m2_{k}_{j}")
                for i in range(NE):
                    nc.tensor.matmul(
                        m2_ps[:],
                        lhsT=wot_t[i][:, 128 * j : 128 * (j + 1)],
                        rhs=attn_t[i][:],
                        start=(i == 0),
                        stop=(i == NE - 1),
                    )
                o_t = outp.tile([128, TC], dt.float32, tag="ot", name=f"ot_{k}_{j}")
                nc.vector.tensor_copy(o_t[:], m2_ps[:])
                nc.sync.dma_start(
                    out_d[128 * j : 128 * (j + 1), TC * k : TC * (k + 1)], o_t[:]
                )

    nc.compile()
    return nc


def _prep_consts(s, z, random_matrices, W_q, b_q, W_out, b_out):
    rm64 = random_matrices.astype(np.float64) / (D ** 0.25)
    wq64 = W_q.astype(np.float64).reshape(H, D, E)  # W_q[h*64+d, e]
    # M[hp, e] = sum_d rm[h,p,d] * W_q[h*64+d, e];  MT = M.T  [e, hp]
    m = np.einsum("hpd,hde->hpe", rm64, wq64).reshape(E, E)
    mt64 = m.T  # [e, hp] fp64
    mtr = tf32_round(mt64.astype(np.float32))
    mte = tf32_round((mt64 - mtr.astype(np.float64)).astype(np.float32))
    assert not b_q.any(), "b_q expected zero (bias path not emitted)"

    wot = tf32_round(np.ascontiguousarray(W_out.T, np.float32))  # [hd, e']

    # s_aug per head: [2P, D+1], cols 0:D = s[b,h]*P**-0.5, col D = z[b,h]*P**-0.5
    scale = P ** -0.5
    saugs = []
    for b in range(B):
        sa = np.zeros((2 * P, H * (D + 1)), np.float32)
        for h in range(H):
            sa[:, (D + 1) * h : (D + 1) * h + D] = s[b, h] * scale
            sa[:, (D + 1) * h + D] = z[b, h] * scale
        saugs.append(sa)

    ones = np.zeros((1, 256), np.float32)
    ones[0, 0:64] = 1.0
    ones[0, 192:256] = 1.0
    ones = tf32_round(ones)
    assert not b_out.any(), "b_out expected zero (bias path not emitted)"
    return mtr, mte, wot, saugs, ones


def kernel(query, s, z, random_matrices, W_q, b_q, W_out, b_out):
    query = np.asarray(query, np.float32)
    s = np.asarray(s, np.float32)
    z = np.asarray(z, np.float32)
    random_matrices = np.asarray(random_matrices, np.float32)
    W_q = np.asarray(W_q, np.float32)
    b_q = np.asarray(b_q, np.float32)
    W_out = np.asarray(W_out, np.float32)
    b_out = np.asarray(b_out, np.float32)

    if "nc" not in _CACHE:
        _CACHE["nc"] = build_kernel()
    nc = _CACHE["nc"]

    mtr, mte, wot, saugs, ones = _prep_consts(
        s, z, random_matrices, W_q, b_q, W_out, b_out
    )

    in_maps = []
    for b in range(NCORES):
        xt = np.ascontiguousarray(query[:, b, :].T)  # [E, T] fp32
        xtr = tf32_round(xt)
        xte = tf32_round(xt - xtr)
        in_maps.append(
            {
                "xtr": xtr,
                "xte": xte,
                "mtr": mtr,
                "mte": mte,
                "wot": wot,
                "saug": saugs[b],
                "ones": ones,
            }
        )

    res = run_bass_kernel_spmd(nc, in_maps, list(range(NCORES)))
    out = np.empty((T, B, E), np.float32)
    for b in range(NCORES):
        out[:, b, :] = res.results[b]["out"].T
    return out



# revision 6
# speedup vs baseline: 3.3137x; 3.3137x over previous
"""Trainium2 Bass kernel for nn_CrossAttention_4037269258775 (RFA cross-attention).

Math (per batch b):
  q   = query @ W_q.T + b_q                  [T, E] -> view [T, H, D]
  wx  = (q / D**0.25) @ rm[h].T              [T, H, P]
  phi = [sin(wx), cos(wx)] * P**-0.5         [T, H, 2P]
  qs  = phi @ s[b,h]; qz = max(phi @ z[b,h], EPS)
  attn = qs / qz                             [T, E]
  out = attn @ W_out.T + b_out               [T, E]

Wall-clock here is dominated by the axon tunnel (~45 MB/s shared), so the
design minimizes host<->device bytes per call:
  - T-sharding: core c gets t-rows [256c, 256(c+1)) for ALL batches, so the
    fp32 query ships as a zero-copy reshape [T, B*E] (67 MB), no host pack.
  - All weight-derived tensors are device-resident across calls, re-uploaded
    only when the weight fingerprint changes.
  - Output returns as bf16 [T, B*E] (34 MB); fp16 would overflow (attn has
    ~1e8 outliers from the EPS clamp on qz).
  - Output buffers (donated) are created on-device, never shipped.

Numerics: a row exists with |qz| ~ 1e-7, and the EPS clamp amplifies any qz
error by ~1e8, so the q-projection uses the error-compensated tf32 scheme:
host precombines M[e, hp] = (rm/D**0.25 · W_q) in fp64, splits M = Mr + Me
(tf32 halves); the device splits X = xtr + xte (xtr = X with low 13 mantissa
bits masked -> tf32-exact; xte = X - xtr) and computes
  wx = Mr@xtr + Mr@xte + Me@xtr   (residual ~2^-21)
Device dataflow is feature-major: fp32 X tiles are PE-transposed (identity
matmul), split on DVE (uint32 shifts + subtract), matmul'd against resident
Mr/Me; sin via range-wrap + ACT Sin; fused qs+qz matmul per head (s_aug has
z as column 64); recip on DVE broadcast across partitions by selector
matmul; out-proj consumes attn tiles as lhsT so the result lands t-major
[t, e'] and DMAs straight into the bf16 [256, B*E] output slice. Biases are
exact via K=1 matmuls (cq row folded from b_q, bout row).
"""
import hashlib
import numpy as np
from contextlib import ExitStack

import concourse.bass as bass
import concourse.tile as tile
import concourse.mybir as mybir
from concourse import bacc
from concourse.bass_utils import run_bass_kernel_spmd  # noqa: F401  (compat)

dt = mybir.dt

T, B, E = 2048, 8, 1024
H, D, P = 16, 64, 64
EPS = 1e-8
NCORES = 8
TPC = T // NCORES             # 256 t-rows per core
NE = E // 128                 # 8 tiles of 128 along e / hp / hd
PI = float(np.pi)
TWO_PI = float(2 * np.pi)
HALF_PI = float(np.pi / 2)

_CACHE = {}


def tf32_round(x):
    u = np.ascontiguousarray(x, np.float32).view(np.uint32)
    r = (u + 0xFFF + ((u >> 13) & 1)) & np.uint32(0xFFFFE000)
    return r.view(np.float32)


def build_kernel():
    nc = bacc.Bacc(None, target_bir_lowering=False)

    x_d = nc.dram_tensor("x", [TPC, B * E], dt.float32, kind="ExternalInput")
    mtr_d = nc.dram_tensor("mtr", [E, E], dt.float32r, kind="ExternalInput")
    mte_d = nc.dram_tensor("mte", [E, E], dt.float32r, kind="ExternalInput")
    wot_d = nc.dram_tensor("wot", [E, E], dt.float32r, kind="ExternalInput")
    saug_d = nc.dram_tensor(
        "saug", [2 * P, B * H * (D + 1)], dt.float32, kind="ExternalInput"
    )
    cq_d = nc.dram_tensor("cq", [1, E], dt.float32r, kind="ExternalInput")
    bout_d = nc.dram_tensor("bout", [1, E], dt.float32r, kind="ExternalInput")
    # pair-broadcast selectors: cols 0:128 = [1]*64+[0]*64, 128:256 = reverse
    ones_d = nc.dram_tensor("ones", [1, 256], dt.float32r, kind="ExternalInput")
    onesr_d = nc.dram_tensor("onesr", [1, TPC], dt.float32r, kind="ExternalInput")
    ident_d = nc.dram_tensor("ident", [128, 128], dt.float32, kind="ExternalInput")
    out_d = nc.dram_tensor("out", [TPC, B * E], dt.bfloat16, kind="ExternalOutput")

    with tile.TileContext(nc) as tc, ExitStack() as ctx:
        consts = ctx.enter_context(tc.tile_pool(name="consts", bufs=1))
        xnp = ctx.enter_context(tc.tile_pool(name="xnp", bufs=2))
        xsp = ctx.enter_context(tc.tile_pool(name="xsp", bufs=1))
        wrp = ctx.enter_context(tc.tile_pool(name="wrp", bufs=2))
        phip = ctx.enter_context(tc.tile_pool(name="phip", bufs=2))
        rcp = ctx.enter_context(tc.tile_pool(name="rcp", bufs=2))
        attnp = ctx.enter_context(tc.tile_pool(name="attnp", bufs=1))
        outp = ctx.enter_context(tc.tile_pool(name="outp", bufs=2))
        ps_tp = ctx.enter_context(tc.tile_pool(name="ps_tp", bufs=1, space="PSUM"))
        ps_wx = ctx.enter_context(tc.tile_pool(name="ps_wx", bufs=2, space="PSUM"))
        ps_qs = ctx.enter_context(tc.tile_pool(name="ps_qs", bufs=1, space="PSUM"))
        ps_bc = ctx.enter_context(tc.tile_pool(name="ps_bc", bufs=1, space="PSUM"))
        ps_m2 = ctx.enter_context(tc.tile_pool(name="ps_m2", bufs=2, space="PSUM"))

        # ---- resident constants ----
        mtr_t = [consts.tile([128, E], dt.float32r, tag=f"mtr{g}", name=f"mtr{g}") for g in range(NE)]
        mte_t = [consts.tile([128, E], dt.float32r, tag=f"mte{g}", name=f"mte{g}") for g in range(NE)]
        wot_t = [consts.tile([128, E], dt.float32r, tag=f"wot{g}", name=f"wot{g}") for g in range(NE)]
        for g in range(NE):
            nc.sync.dma_start(mtr_t[g][:], mtr_d[128 * g : 128 * (g + 1), :])
            nc.sync.dma_start(mte_t[g][:], mte_d[128 * g : 128 * (g + 1), :])
            nc.sync.dma_start(wot_t[g][:], wot_d[128 * g : 128 * (g + 1), :])
        saug_t = consts.tile([2 * P, B * H * (D + 1)], dt.float32, tag="saug", name="saug")
        nc.sync.dma_start(saug_t[:], saug_d[:])
        cq_t = consts.tile([1, E], dt.float32r, tag="cq", name="cq")
        nc.sync.dma_start(cq_t[:], cq_d[:])
        bout_t = consts.tile([1, E], dt.float32r, tag="bout", name="bout")
        nc.sync.dma_start(bout_t[:], bout_d[:])
        ones_t = consts.tile([1, 256], dt.float32r, tag="ones", name="ones")
        nc.sync.dma_start(ones_t[:], ones_d[:])
        onesr_t = consts.tile([1, TPC], dt.float32r, tag="onesr", name="onesr")
        nc.sync.dma_start(onesr_t[:], onesr_d[:])
        ident_t = consts.tile([128, 128], dt.float32, tag="ident", name="ident")
        nc.sync.dma_start(ident_t[:], ident_d[:])

        for b in range(B):
            # ---- load natural [t, e] tiles for batch b ----
            xn_t = []
            for tt in range(2):
                xn = xnp.tile([128, E], dt.float32, tag=f"xn{tt}", name=f"xn_{b}_{tt}")
                nc.sync.dma_start(
                    xn[:], x_d[128 * tt : 128 * (tt + 1), E * b : E * (b + 1)]
                )
                xn_t.append(xn)
            # ---- PE-transpose to [e, t]; split into tf32-exact xtr + xte ----
            xtr_t, xte_t = [], []
            for g in range(NE):
                tr = xsp.tile([128, TPC], dt.float32r, tag=f"xtr{g}", name=f"xtr_{b}_{g}")
                te = xsp.tile([128, TPC], dt.float32r, tag=f"xte{g}", name=f"xte_{b}_{g}")
                for tt in range(2):
                    tp_ps = ps_tp.tile([128, 128], dt.float32, tag="tp", name=f"tp_{b}_{g}_{tt}")
                    nc.tensor.transpose(
                        tp_ps[:], xn_t[tt][:, 128 * g : 128 * (g + 1)], ident_t[:]
                    )
                    sl = slice(128 * tt, 128 * (tt + 1))
                    # f32r writes round to the PE's reduced precision, so
                    # xtr is matmul-exact and xte captures the residual.
                    nc.vector.tensor_copy(tr[:, sl], tp_ps[:])
                    nc.vector.tensor_tensor(
                        te[:, sl], tp_ps[:], tr[:, sl], op=mybir.AluOpType.subtract
                    )
                xtr_t.append(tr)
                xte_t.append(te)

            attn_t = []
            for i in range(NE):  # hp-tile i: heads 2i (parts 0:64), 2i+1 (64:128)
                # ---- wx = M @ X^T, 3-term compensated tf32 ----
                wx_ps = ps_wx.tile([128, TPC], dt.float32, tag="wx", name=f"wx_{b}_{i}")
                mi = 0
                for mg, xg in ((mtr_t, xtr_t), (mtr_t, xte_t), (mte_t, xtr_t)):
                    for g in range(NE):
                        nc.tensor.matmul(
                            wx_ps[:],
                            lhsT=mg[g][:, 128 * i : 128 * (i + 1)],
                            rhs=xg[g][:],
                            start=(mi == 0),
                            stop=False,
                        )
                        mi += 1
                # exact b_q bias row (zero in practice): cq slice x ones-row
                nc.tensor.matmul(
                    wx_ps[:],
                    lhsT=cq_t[:, 128 * i : 128 * (i + 1)],
                    rhs=onesr_t[:],
                    start=False,
                    stop=True,
                )
                # ---- range reduction into [-pi, pi] ----
                wr_a = wrp.tile([128, TPC], dt.float32, tag="wr_a", name=f"wra_{b}_{i}")
                nc.vector.add_range_wrap(wr_a[:], wx_ps[:], 0.0, PI, TWO_PI)
                wr_s = wrp.tile([128, TPC], dt.float32, tag="wr_s", name=f"wrs_{b}_{i}")
                nc.vector.add_range_wrap(wr_s[:], wr_a[:], 0.0, PI, TWO_PI)
                wr_c = wrp.tile([128, TPC], dt.float32, tag="wr_c", name=f"wrc_{b}_{i}")
                nc.vector.add_range_wrap(wr_c[:], wr_s[:], HALF_PI, PI, TWO_PI)

                ph = []
                for half in range(2):
                    phi_t = phip.tile(
                        [128, TPC], dt.float32, tag=f"phi{half}", name=f"phi_{b}_{i}_{half}"
                    )
                    sl = slice(64 * half, 64 * (half + 1))
                    nc.scalar.activation(
                        phi_t[0:64, :], wr_s[sl, :], mybir.ActivationFunctionType.Sin
                    )
                    nc.scalar.activation(
                        phi_t[64:128, :], wr_c[sl, :], mybir.ActivationFunctionType.Sin
                    )
                    ph.append(phi_t)

                attn_i = attnp.tile(
                    [128, TPC], dt.float32r, tag=f"attn{i}", name=f"attn_{b}_{i}"
                )
                qs_pair = []
                rcr = [
                    rcp.tile([1, TPC], dt.float32r, tag="rcr0", name=f"rcr0_{b}_{i}"),
                    rcp.tile([1, TPC], dt.float32r, tag="rcr1", name=f"rcr1_{b}_{i}"),
                ]
                for half in range(2):
                    h = 2 * i + half
                    # ---- fused qs+qz fp32 matmul: s_aug col block [128, 65] ----
                    qs_ps = ps_qs.tile(
                        [65, TPC], dt.float32, tag=f"qs{half}", name=f"qs_{b}_{h}"
                    )
                    co = (b * H + h) * (D + 1)
                    nc.tensor.matmul(
                        qs_ps[:],
                        lhsT=saug_t[:, co : co + D + 1],
                        rhs=ph[half][:],
                        start=True,
                        stop=True,
                    )
                    qs_pair.append(qs_ps)
                    qz_c = rcp.tile([1, TPC], dt.float32, tag="qz_c", name=f"qzc_{b}_{h}", bufs=1)
                    nc.vector.tensor_scalar_max(qz_c[:], qs_ps[64:65, :], EPS)
                    rc32 = rcp.tile([1, TPC], dt.float32, tag="rc32", name=f"rc32_{b}_{h}", bufs=1)
                    nc.vector.reciprocal(rc32[:], qz_c[:])
                    nc.vector.tensor_copy(rcr[half][:], rc32[:])
                # ---- broadcast recips across partitions via selector matmuls ----
                bc_ps = ps_bc.tile([128, TPC], dt.float32, tag="bc", name=f"bc_{b}_{i}")
                nc.tensor.matmul(
                    bc_ps[:], lhsT=ones_t[:, 0:128], rhs=rcr[0][:], start=True, stop=False
                )
                nc.tensor.matmul(
                    bc_ps[:], lhsT=ones_t[:, 128:256], rhs=rcr[1][:], start=False, stop=True
                )
                bc_sb = rcp.tile([128, TPC], dt.float32, tag="bc_sb", name=f"bcs_{b}_{i}")
                nc.vector.tensor_copy(bc_sb[:], bc_ps[:])
                for half in range(2):
                    nc.vector.tensor_mul(
                        attn_i[64 * half : 64 * (half + 1), :],
                        qs_pair[half][0:64, :],
                        bc_sb[64 * half : 64 * (half + 1), :],
                    )
                attn_t.append(attn_i)

            # ---- out projection, t-major: out[t, e'] = attn.T^T @ wot + b_out ----
            for tt in range(2):
                tsl = slice(128 * tt, 128 * (tt + 1))
                for j in range(4):
                    m2_ps = ps_m2.tile([128, 256], dt.float32, tag="m2", name=f"m2_{b}_{tt}_{j}")
                    for i in range(NE):
                        nc.tensor.matmul(
                            m2_ps[:],
                            lhsT=attn_t[i][:, tsl],
                            rhs=wot_t[i][:, 256 * j : 256 * (j + 1)],
                            start=(i == 0),
                            stop=False,
                        )
                    nc.tensor.matmul(
                        m2_ps[:],
                        lhsT=onesr_t[:, 0:128],
                        rhs=bout_t[:, 256 * j : 256 * (j + 1)],
                        start=False,
                        stop=True,
                    )
                    o_t = outp.tile([128, 256], dt.bfloat16, tag="ot", name=f"ot_{b}_{tt}_{j}")
                    nc.vector.tensor_copy(o_t[:], m2_ps[:])
                    nc.sync.dma_start(
                        out_d[tsl, E * b + 256 * j : E * b + 256 * (j + 1)], o_t[:]
                    )

    nc.compile()
    return nc


def _prep_consts(s, z, random_matrices, W_q, b_q, W_out, b_out):
    rm64 = random_matrices.astype(np.float64) / (D ** 0.25)
    wq64 = W_q.astype(np.float64).reshape(H, D, E)  # W_q[h*64+d, e]
    m = np.einsum("hpd,hde->hpe", rm64, wq64).reshape(E, E)
    mt64 = m.T  # [e, hp] fp64
    mtr = tf32_round(mt64.astype(np.float32))
    mte = tf32_round((mt64 - mtr.astype(np.float64)).astype(np.float32))

    wot = tf32_round(np.ascontiguousarray(W_out.T, np.float32))  # [hd, e']

    scale = P ** -0.5
    saug = np.zeros((2 * P, B * H * (D + 1)), np.float32)
    for b in range(B):
        for h in range(H):
            co = (b * H + h) * (D + 1)
            saug[:, co : co + D] = s[b, h] * scale
            saug[:, co + D] = z[b, h] * scale

    # cq[hp] = sum_d rm64[h,p,d] * b_q[h*64+d]
    cq = np.einsum("hpd,hd->hp", rm64, b_q.astype(np.float64).reshape(H, D))
    cq = tf32_round(cq.reshape(1, E).astype(np.float32))
    bout = tf32_round(b_out.astype(np.float32).reshape(1, E))

    ones = np.zeros((1, 256), np.float32)
    ones[0, 0:64] = 1.0
    ones[0, 192:256] = 1.0
    onesr = np.ones((1, TPC), np.float32)
    ident = np.eye(128, dtype=np.float32)
    return {
        "mtr": mtr, "mte": mte, "wot": wot, "saug": saug,
        "cq": cq, "bout": bout, "ones": ones, "onesr": onesr, "ident": ident,
    }


def _weights_fingerprint(*arrs):
    hsh = hashlib.blake2b(digest_size=16)
    for a in arrs:
        hsh.update(np.ascontiguousarray(a).tobytes())
    return hsh.hexdigest()


def _get_state():
    if "st" in _CACHE:
        return _CACHE["st"]

    import jax
    import jax.numpy as jnp
    from jax.sharding import Mesh, PartitionSpec, NamedSharding
    from jax.experimental.shard_map import shard_map
    from concourse.bass2jax import (
        _bass_exec_p,
        install_neuronx_cc_hook,
        partition_id_tensor,
    )

    nc = build_kernel()
    install_neuronx_cc_hook()

    partition_name = nc.partition_id_tensor.name if nc.partition_id_tensor else None
    in_names, out_names, out_avals = [], [], []
    for alloc in nc.m.functions[0].allocations:
        if not isinstance(alloc, mybir.MemoryLocationSet):
            continue
        name = alloc.memorylocations[0].name
        if alloc.kind == "ExternalInput":
            if name != partition_name:
                in_names.append(name)
        elif alloc.kind == "ExternalOutput":
            out_names.append(name)
            out_avals.append(
                jax.core.ShapedArray(tuple(alloc.tensor_shape), dt.np(alloc.dtype))
            )
    n_params = len(in_names)
    all_names = in_names + out_names
    if partition_name is not None:
        all_names = all_names + [partition_name]

    def _body(*args):
        operands = list(args)
        if partition_name is not None:
            operands.append(partition_id_tensor())
        outs = _bass_exec_p.bind(
            *operands,
            out_avals=tuple(out_avals),
            in_names=tuple(all_names),
            out_names=tuple(out_names),
            lowering_input_output_aliases=(),
            sim_require_finite=True,
            sim_require_nnan=True,
            nc=nc,
        )
        return tuple(outs)

    devices = jax.devices()[:NCORES]
    mesh = Mesh(np.asarray(devices), ("core",))
    shard = NamedSharding(mesh, PartitionSpec("core"))
    n_outs = len(out_names)
    sharded = jax.jit(
        shard_map(
            _body,
            mesh=mesh,
            in_specs=(PartitionSpec("core"),) * (n_params + n_outs),
            out_specs=(PartitionSpec("core"),) * n_outs,
            check_rep=False,
        ),
        donate_argnums=tuple(range(n_params, n_params + n_outs)),
        keep_unused=True,
    )
    mk_zeros = jax.jit(
        lambda: jnp.zeros((T, B * E), jnp.bfloat16), out_shardings=shard
    )

    st = {
        "jax": jax,
        "nc": nc,
        "in_names": in_names,
        "sharded": sharded,
        "mk_zeros": mk_zeros,
        "shard": shard,
        "wfp": None,
        "wdev": None,
    }
    _CACHE["st"] = st
    return st


def kernel(query, s, z, random_matrices, W_q, b_q, W_out, b_out):
    query = np.ascontiguousarray(query, np.float32)
    s = np.asarray(s, np.float32)
    z = np.asarray(z, np.float32)
    random_matrices = np.asarray(random_matrices, np.float32)
    W_q = np.asarray(W_q, np.float32)
    b_q = np.asarray(b_q, np.float32)
    W_out = np.asarray(W_out, np.float32)
    b_out = np.asarray(b_out, np.float32)

    st = _get_state()
    jax = st["jax"]

    wfp = _weights_fingerprint(s, z, random_matrices, W_q, b_q, W_out, b_out)
    if st["wfp"] != wfp:
        consts = _prep_consts(s, z, random_matrices, W_q, b_q, W_out, b_out)
        wdev = {}
        for name, arr in consts.items():
            glob = np.tile(arr, (NCORES, 1))
            wdev[name] = jax.device_put(glob, st["shard"])
        for d in wdev.values():
            d.block_until_ready()
        st["wdev"] = wdev
        st["wfp"] = wfp

    xg = jax.device_put(query.reshape(T, B * E), st["shard"])
    zs = st["mk_zeros"]()
    args = [xg if nm == "x" else st["wdev"][nm] for nm in st["in_names"]]
    (outg,) = st["sharded"](*args, zs)
    res = np.asarray(outg)  # [T, B*E] bf16
    return res.astype(np.float32).reshape(T, B, E)
